# revision 1
# baseline (speedup 1.0000x reference)
"""Trainium2 Bass kernel for HCEN forward: out = ((x.mean(axis=1)) @ W_enc.T + b_enc) @ W_out.T + b_out.

Sharding: data-parallel over batch. B=16 across 8 cores -> 2 batches/core
(32 MB of x each). Weights replicated per core (host pre-transposed so the
contraction dim lands on partitions). No collectives needed.

Per-core pipeline (final, ~118 us; x-stream runs at ~390 GB/s, near the
~358 GB/s per-core HBM roofline):
  phase 1: stream x in [128, 4, 1024] tiles (2 MB DMAs); 4 DVE adds per tile
           accumulate directly into acc[128, 1024] per batch (no fold tail).
  phase 1b: 8 ones-matmuls per batch ([128s,128d]^T @ ones -> mT[d,1], f32),
           scaled 1/S on the ACT copy out of PSUM -> mt_sb[128, c, b] (bf16).
  layer 1: bf16, M=2 orientation (single PE pass at N=512 vs 2 passes for
           f32): stationary mT [128,2], moving W_encT chunks [128,512] ->
           enc[2,1024] f32 PSUM; bias folded into the PSUM->SBUF move as a
           DVE add against a partition-broadcast bias tile.
  transpose: enc -> encT tiles [128,2] via PE transpose (ident2).
  layer 2: same bf16 M=2 form -> out[2,1024] + DVE bias add.
  out: [2, 1024] per core, natural layout; host concatenates.
  Weights ship as host-converted bf16 (halves their DMA bytes) in 8 chunk
  DMAs each, queued after x so the x critical path drains first while
  layer-1 can start on early chunks.
"""

import os
import sys
from contextlib import ExitStack

import ml_dtypes
import numpy as np

for _p in ("/opt/trn_rl_repo", "/root/.axon_site/_ro/trn_rl_repo"):
    if os.path.isdir(_p) and _p not in sys.path:
        sys.path.insert(0, _p)

import concourse.bass as bass  # noqa: E402
import concourse.tile as tile  # noqa: E402
from concourse import bacc, mybir  # noqa: E402
from concourse.bass_utils import run_bass_kernel_spmd  # noqa: E402
from concourse.masks import make_identity  # noqa: E402

B, S, D, H, O = 16, 4096, 1024, 1024, 1024
NCORES = 8
BPC = B // NCORES  # batches per core
P = 128
QT = 4  # s-subtiles per DMA tile -> [128, QT*1024] = 2 MB
NT = S // (P * QT)  # DMA tiles per batch
DC = D // P
HC = H // P
OC = O // P
NF = 512  # matmul moving free dim (PSUM bank limit)
F32 = mybir.dt.float32
BF16 = mybir.dt.bfloat16

_CACHE = {}


def build_nc():
    if "nc" in _CACHE:
        return _CACHE["nc"]
    nc = bacc.Bacc(
        "TRN2",
        target_bir_lowering=False,
        debug=False,
        enable_asserts=False,
        num_devices=NCORES,
    )
    x_ext = nc.dram_tensor("x", [BPC, S, D], F32, kind="ExternalInput").ap()
    wencT_ext = nc.dram_tensor("wencT", [D, H], BF16, kind="ExternalInput").ap()
    woutT_ext = nc.dram_tensor("woutT", [H, O], BF16, kind="ExternalInput").ap()
    benc_ext = nc.dram_tensor("benc", [H], F32, kind="ExternalInput").ap()
    bout_ext = nc.dram_tensor("bout", [O], F32, kind="ExternalInput").ap()
    out_ext = nc.dram_tensor("out", [BPC, O], F32, kind="ExternalOutput").ap()

    with ExitStack() as ctx:
        tc = ctx.enter_context(tile.TileContext(nc))
        consts = ctx.enter_context(tc.tile_pool(name="consts", bufs=1))
        wpool = ctx.enter_context(tc.tile_pool(name="wpool", bufs=1))
        xpool = ctx.enter_context(tc.tile_pool(name="xpool", bufs=4))
        apool = ctx.enter_context(tc.tile_pool(name="apool", bufs=1))
        spool = ctx.enter_context(tc.tile_pool(name="spool", bufs=1))
        mtp = ctx.enter_context(tc.tile_pool(name="mtp", bufs=2, space="PSUM"))
        pp2 = ctx.enter_context(tc.tile_pool(name="pp2", bufs=1, space="PSUM"))
        tpp = ctx.enter_context(tc.tile_pool(name="tpp", bufs=2, space="PSUM"))

        ones_sb = consts.tile([P, 1], F32)
        nc.gpsimd.memset(ones_sb[:], 1.0)
        ident2 = consts.tile([BPC, BPC], F32)
        make_identity(nc, ident2[:])

        # phase 1: stream x; per tile, 4 DVE adds into acc[128, 1024]
        mt_sb = spool.tile([P, DC, BPC], BF16)
        accs = [
            apool.tile([P, D], F32, name=f"acc{b}", tag=f"acc{b}") for b in range(BPC)
        ]
        for b in range(BPC):
            for t in range(NT):
                xt = xpool.tile([P, QT, D], F32, name="xt", tag="xt")
                nc.sync.dma_start(
                    xt[:],
                    x_ext[b, t * P * QT : (t + 1) * P * QT, :].rearrange(
                        "(q p) d -> p q d", p=P
                    ),
                )
                for q in range(QT):
                    if t == 0 and q == 0:
                        nc.vector.tensor_copy(accs[b][:], xt[:, 0, :])
                    else:
                        nc.vector.tensor_add(accs[b][:], accs[b][:], xt[:, q, :])
            for c in range(DC):
                mt_ps = mtp.tile([P, 1], F32, name=f"mt_ps{b}_{c}", tag="mtps")
                nc.tensor.matmul(mt_ps[:], accs[b][:, c * P : (c + 1) * P], ones_sb[:])
                nc.scalar.mul(mt_sb[:, c, b : b + 1], mt_ps[:], 1.0 / S)

        # weights: 8 x 512 KB chunk DMAs each, after x in program order
        wenc_sb = wpool.tile([P, DC, H], BF16)
        for c in range(DC):
            nc.sync.dma_start(
                wenc_sb[:, c, :], wencT_ext[c * P : (c + 1) * P, :]
            )
        wout_sb = wpool.tile([P, HC, O], BF16)
        for c in range(HC):
            nc.sync.dma_start(
                wout_sb[:, c, :], woutT_ext[c * P : (c + 1) * P, :]
            )

        benc2 = consts.tile([BPC, H], F32, name="benc2")
        nc.sync.dma_start(benc2[:], benc_ext[None, :].broadcast_to([BPC, H]))
        bout2 = consts.tile([BPC, O], F32, name="bout2")
        nc.sync.dma_start(bout2[:], bout_ext[None, :].broadcast_to([BPC, O]))

        # layer 1 (bf16): enc[2, 1024] = mT.T @ W_encT + b_enc
        enc_ps = pp2.tile([BPC, H], F32, name="enc_ps", tag="eps")
        enc_sb = spool.tile([BPC, H], F32)
        for n in range(H // NF):
            sl = slice(n * NF, (n + 1) * NF)
            for c in range(DC):
                nc.tensor.matmul(
                    enc_ps[:, sl],
                    mt_sb[:, c, :],
                    wenc_sb[:, c, sl],
                    start=(c == 0),
                    stop=(c == DC - 1),
                )
            nc.vector.tensor_add(enc_sb[:, sl], enc_ps[:, sl], benc2[:, sl])

        # transpose enc -> encT tiles [128, 2]
        encT_sb = spool.tile([P, HC, BPC], BF16)
        for c in range(HC):
            tp = tpp.tile([P, BPC], F32, name=f"tp{c}", tag="tps")
            nc.tensor.transpose(tp[:], enc_sb[:, c * P : (c + 1) * P], ident2[:])
            nc.scalar.copy(encT_sb[:, c, :], tp[:])

        # layer 2 (bf16): out[2, 1024] = encT.T @ W_outT + b_out
        out_ps = pp2.tile([BPC, O], F32, name="out_ps", tag="ops")
        out_sb = spool.tile([BPC, O], F32)
        for n in range(O // NF):
            sl = slice(n * NF, (n + 1) * NF)
            for c in range(HC):
                nc.tensor.matmul(
                    out_ps[:, sl],
                    encT_sb[:, c, :],
                    wout_sb[:, c, sl],
                    start=(c == 0),
                    stop=(c == HC - 1),
                )
            nc.vector.tensor_add(out_sb[:, sl], out_ps[:, sl], bout2[:, sl])
        nc.sync.dma_start(out_ext[:], out_sb[:])

    nc.compile()
    _CACHE["nc"] = nc
    return nc


def make_in_maps(x, W_enc, b_enc, W_out, b_out):
    x = np.ascontiguousarray(np.asarray(x, dtype=np.float32))
    wencT = np.ascontiguousarray(np.asarray(W_enc, dtype=np.float32).T.astype(ml_dtypes.bfloat16))
    woutT = np.ascontiguousarray(np.asarray(W_out, dtype=np.float32).T.astype(ml_dtypes.bfloat16))
    benc = np.ascontiguousarray(np.asarray(b_enc, dtype=np.float32))
    bout = np.ascontiguousarray(np.asarray(b_out, dtype=np.float32))
    return [
        {
            "x": x[i * BPC : (i + 1) * BPC],
            "wencT": wencT,
            "woutT": woutT,
            "benc": benc,
            "bout": bout,
        }
        for i in range(NCORES)
    ]


def gather_out(results):
    return np.ascontiguousarray(
        np.concatenate([results[i]["out"] for i in range(NCORES)], axis=0)
    )


def kernel(x, W_enc, b_enc, W_out, b_out):
    nc = build_nc()
    in_maps = make_in_maps(x, W_enc, b_enc, W_out, b_out)
    res = run_bass_kernel_spmd(nc, in_maps, list(range(NCORES)))
    return gather_out(res.results)



# revision 3
# speedup vs baseline: 1.5892x; 1.5892x over previous
"""Trainium2 Bass kernel for HCEN forward: out = ((x.mean(axis=1)) @ W_enc.T + b_enc) @ W_out.T + b_out.

Sharding: data-parallel over batch. B=16 across 8 cores -> 2 batches/core.
Weights replicated per core. No collectives.

Key ideas (network is fully linear in x, tolerance 2e-2):
  * x ships as bf16 (host cast) -> halves the dominant HBM stream
    (32 -> 16 MiB/core). Mean-of-4096 washes the rounding error out
    (~0.4% << 2e-2).
  * The two layers collapse: out = m @ C + bias_c with
    C = W_enc.T @ W_out.T and bias_c = b_enc @ W_out.T + b_out.
    C (1024x1024) is built ON DEVICE by the otherwise-idle PE engine
    while x streams (also keeps the PE HAM clock-gate warm), so the
    post-stream tail is just: last fold + ones-matmul + one 16-matmul
    pass (mT.T @ C) + bias row via a K=1 ones-matmul + out DMA.
  * Mean pipeline per 2 MiB x tile: DVE tree-fold in bf16
    (free-dim 4096/2048/1024 adds), then one mixed add into an f32 acc.
    Per batch: acc -> bf16, then 8 stationary-acc ones-matmuls produce
    mT [128, 8] directly in PSUM (bf16 single pass, no transposes).
  * Weight DMAs are interleaved early into the x stream (after 2 x
    tiles) instead of queued behind all of x.
  * Engine FIFO issue order is arranged so the PE queue is:
    C(n=0) | b0-mt | C(n=1) | bias | b1-mt | final -- C paces the PE
    through the stream and the tail only contains b1-mt + final.
"""

import os
import sys
from contextlib import ExitStack

import ml_dtypes
import numpy as np

for _p in ("/opt/trn_rl_repo", "/root/.axon_site/_ro/trn_rl_repo"):
    if os.path.isdir(_p) and _p not in sys.path:
        sys.path.insert(0, _p)

import concourse.bass as bass  # noqa: E402
import concourse.tile as tile  # noqa: E402
from concourse import bacc, mybir  # noqa: E402
from concourse.bass_utils import run_bass_kernel_spmd  # noqa: E402

B, S, D, H, O = 16, 4096, 1024, 1024, 1024
NCORES = 8
BPC = B // NCORES  # batches per core
P = 128
QT = 8  # s-subtiles per DMA tile -> [128, QT, 1024] bf16 = 2 MiB
NT = S // (P * QT)  # DMA tiles per batch (4)
DC = D // P
HC = H // P
NF = 512  # matmul moving free dim (PSUM bank limit)
F32 = mybir.dt.float32
BF16 = mybir.dt.bfloat16

_CACHE = {}


def build_nc():
    if "nc" in _CACHE:
        return _CACHE["nc"]
    nc = bacc.Bacc(
        "TRN2",
        target_bir_lowering=False,
        debug=False,
        enable_asserts=False,
        num_devices=NCORES,
    )
    x_ext = nc.dram_tensor("x", [BPC, S, D], BF16, kind="ExternalInput").ap()
    wenc_ext = nc.dram_tensor("wenc", [H, D], BF16, kind="ExternalInput").ap()
    woutT_ext = nc.dram_tensor("woutT", [H, O], BF16, kind="ExternalInput").ap()
    bencT_ext = nc.dram_tensor("bencT", [P, HC], BF16, kind="ExternalInput").ap()
    bout_ext = nc.dram_tensor("bout", [1, O], F32, kind="ExternalInput").ap()
    out_ext = nc.dram_tensor("out", [BPC, O], F32, kind="ExternalOutput").ap()

    with ExitStack() as ctx:
        tc = ctx.enter_context(tile.TileContext(nc))
        consts = ctx.enter_context(tc.tile_pool(name="consts", bufs=1))
        wpool = ctx.enter_context(tc.tile_pool(name="wpool", bufs=1))
        xpool = ctx.enter_context(tc.tile_pool(name="xpool", bufs=5))
        apool = ctx.enter_context(tc.tile_pool(name="apool", bufs=1))
        spool = ctx.enter_context(tc.tile_pool(name="spool", bufs=1))
        cpp = ctx.enter_context(tc.tile_pool(name="cpp", bufs=2, space="PSUM"))
        mtp = ctx.enter_context(tc.tile_pool(name="mtp", bufs=2, space="PSUM"))
        bpp = ctx.enter_context(tc.tile_pool(name="bpp", bufs=1, space="PSUM"))
        opp = ctx.enter_context(tc.tile_pool(name="opp", bufs=1, space="PSUM"))

        ones_bf = consts.tile([P, 1], BF16)
        nc.gpsimd.memset(ones_bf[:], 1.0)
        ones2_bf = consts.tile([1, BPC], BF16)
        nc.gpsimd.memset(ones2_bf[:], 1.0)

        # ---- DMA program order: 2 x tiles, all weights, rest of x ----
        xts = [[None] * NT for _ in range(BPC)]

        def issue_x(b, t):
            xt = xpool.tile([P, QT, D], BF16, name="xt", tag="xt")
            nc.sync.dma_start(
                xt[:],
                x_ext[b, t * P * QT : (t + 1) * P * QT, :].rearrange(
                    "(p q) d -> p q d", p=P
                ),
            )
            xts[b][t] = xt

        issue_x(0, 0)
        issue_x(0, 1)

        wenc_sb = wpool.tile([P, HC, D], BF16)
        for c in range(HC):
            nc.sync.dma_start(wenc_sb[:, c, :], wenc_ext[c * P : (c + 1) * P, :])
        wout_sb = wpool.tile([P, HC, O], BF16)
        for c in range(HC):
            nc.sync.dma_start(wout_sb[:, c, :], woutT_ext[c * P : (c + 1) * P, :])
        bencT_sb = consts.tile([P, HC], BF16)
        nc.sync.dma_start(bencT_sb[:], bencT_ext[:])
        bout_sb = consts.tile([1, O], F32)
        nc.sync.dma_start(bout_sb[:], bout_ext[:])

        issue_x(0, 2)
        issue_x(0, 3)
        for t in range(NT):
            issue_x(1, t)

        # ---- per-batch mean pipeline pieces ----
        accs = [apool.tile([P, D], F32, name=f"acc{b}") for b in range(BPC)]
        acc_bfs = [apool.tile([P, D], BF16, name=f"accbf{b}") for b in range(BPC)]
        mt_sb = spool.tile([P, DC, BPC], BF16)

        def issue_folds(b):
            # DVE: tree-fold each tile in bf16, then mixed add into f32 acc
            for t in range(NT):
                xt = xts[b][t]
                nc.vector.tensor_add(xt[:, 0:4, :], xt[:, 0:4, :], xt[:, 4:8, :])
                nc.vector.tensor_add(xt[:, 0:2, :], xt[:, 0:2, :], xt[:, 2:4, :])
                nc.vector.tensor_add(xt[:, 0, :], xt[:, 0, :], xt[:, 1, :])
                if t == 0:
                    nc.vector.tensor_copy(accs[b][:], xt[:, 0, :])
                else:
                    nc.vector.tensor_add(accs[b][:], accs[b][:], xt[:, 0, :])
            nc.vector.tensor_copy(acc_bfs[b][:], accs[b][:])

        def issue_mt(b):
            # PE: mT[d, b] = column sums of acc via stationary-acc ones-matmul
            mt_ps = mtp.tile([P, DC], F32, name=f"mtps{b}", tag="mtps")
            for c in range(DC):
                nc.tensor.matmul(
                    mt_ps[:, c : c + 1],
                    acc_bfs[b][:, c * P : (c + 1) * P],
                    ones_bf[:],
                    skip_group_check=True,
                )
            nc.scalar.mul(mt_sb[:, :, b], mt_ps[:], 1.0 / S)

        def issue_c_half(n):
            # PE: C[:, n*NF:(n+1)*NF] = (W_enc.T @ W_out.T) slice (bf16)
            for d in range(DC):
                c_ps = cpp.tile([P, NF], F32, name=f"cps{n}_{d}", tag="cps")
                for h in range(HC):
                    nc.tensor.matmul(
                        c_ps[:],
                        wenc_sb[:, h, d * P : (d + 1) * P],
                        wout_sb[:, h, n * NF : (n + 1) * NF],
                        start=(h == 0),
                        stop=(h == HC - 1),
                    )
                nc.scalar.copy(c_sb[:, d, n * NF : (n + 1) * NF], c_ps[:])

        c_sb = wpool.tile([P, DC, O], BF16)
        bias_ps = bpp.tile([1, O], F32, name="biasps")
        bias_sb = spool.tile([1, O], BF16)

        issue_folds(0)
        issue_c_half(0)
        issue_mt(0)
        issue_c_half(1)
        # PE: bias_c = b_enc @ W_out.T (accumulated in PSUM)
        for n in range(O // NF):
            for h in range(HC):
                nc.tensor.matmul(
                    bias_ps[:, n * NF : (n + 1) * NF],
                    bencT_sb[:, h : h + 1],
                    wout_sb[:, h, n * NF : (n + 1) * NF],
                    start=(h == 0),
                    stop=(h == HC - 1),
                )
        issue_folds(1)
        # bias_sb add issued after b1 folds so it doesn't block the DVE FIFO
        nc.vector.tensor_add(bias_sb[:], bias_ps[:], bout_sb[:])
        issue_mt(1)

        # ---- tail: out = mT.T @ C + ones2.T @ bias_c ----
        out_ps = opp.tile([BPC, O], F32, name="outps")
        out_sb = spool.tile([BPC, O], F32)
        for n in range(O // NF):
            sl = slice(n * NF, (n + 1) * NF)
            for c in range(DC):
                nc.tensor.matmul(
                    out_ps[:, sl],
                    mt_sb[:, c, :],
                    c_sb[:, c, sl],
                    start=(c == 0),
                    stop=False,
                )
            nc.tensor.matmul(
                out_ps[:, sl],
                ones2_bf[:],
                bias_sb[:, sl],
                start=False,
                stop=True,
            )
            nc.scalar.copy(out_sb[:, sl], out_ps[:, sl])
        nc.sync.dma_start(out_ext[:], out_sb[:])

    nc.compile()
    _CACHE["nc"] = nc
    return nc


def make_in_maps(x, W_enc, b_enc, W_out, b_out):
    xb = np.asarray(x, dtype=np.float32).astype(ml_dtypes.bfloat16)
    wenc = np.ascontiguousarray(
        np.asarray(W_enc, dtype=np.float32).astype(ml_dtypes.bfloat16)
    )
    woutT = np.ascontiguousarray(
        np.asarray(W_out, dtype=np.float32).T.astype(ml_dtypes.bfloat16)
    )
    bencT = np.ascontiguousarray(
        np.asarray(b_enc, dtype=np.float32).reshape(HC, P).T.astype(ml_dtypes.bfloat16)
    )
    bout = np.ascontiguousarray(np.asarray(b_out, dtype=np.float32).reshape(1, O))
    return [
        {
            "x": np.ascontiguousarray(xb[i * BPC : (i + 1) * BPC]),
            "wenc": wenc,
            "woutT": woutT,
            "bencT": bencT,
            "bout": bout,
        }
        for i in range(NCORES)
    ]


def gather_out(results):
    return np.ascontiguousarray(
        np.concatenate([results[i]["out"] for i in range(NCORES)], axis=0)
    )


def kernel(x, W_enc, b_enc, W_out, b_out):
    nc = build_nc()
    in_maps = make_in_maps(x, W_enc, b_enc, W_out, b_out)
    res = run_bass_kernel_spmd(nc, in_maps, list(range(NCORES)))
    return gather_out(res.results)
